# revision 1
# baseline (speedup 1.0000x reference)
"""Trainium2 Bass kernel for nn_Encoder_24283745092038 (4-layer dense transformer).

Sharding: data-parallel over batch. 8 cores x 2 sequences each, no collectives.
Per core, activations live feature-major in SBUF: hT[d_part, d_outer, token],
dtype float32r (TF32-class matmul precision at full PE speed, ~1.3e-4/matmul).

kernel(**inputs) takes the FULL unsharded inputs and returns FULL [16, 3] logits.
"""
import os
import numpy as np
import ml_dtypes

import concourse.bass as bass
import concourse.bacc as bacc
import concourse.mybir as mybir
import concourse.tile as tile
from concourse.bass_utils import run_bass_kernel_spmd
from concourse.masks import make_identity

P = 128
V, D, T, H, NC = 50257, 1024, 512, 16, 3
HS = D // H          # 64
HE = H * HS          # 1024
DFF = 4 * D          # 4096
B = 16
NCORES = 8
BLOC = B // NCORES   # 2 sequences per core
TLOC = BLOC * T      # 1024 tokens per core
DO = D // P          # 8 d-chunks
HO = HE // P         # 8 he-chunks
FO = DFF // P        # 32 f-chunks
NLAYERS = int(os.environ.get("NN_LAYERS", "4"))
TAPS = bool(int(os.environ.get("NN_TAPS", "0")))

F32 = mybir.dt.float32
F32R = mybir.dt.float32r
BF16 = mybir.dt.bfloat16
AF = mybir.ActivationFunctionType
OP = mybir.AluOpType

LAST_RESULTS = None  # stash for test.py


def _build(nlayers=NLAYERS, taps=TAPS):
    nc = bacc.Bacc("TRN2", target_bir_lowering=False, debug=False,
                   num_devices=NCORES)

    # ---- DRAM I/O ----------------------------------------------------------
    x_idx = nc.dram_tensor("x_idx", [TLOC, 1], mybir.dt.int32, kind="ExternalInput")
    tok_emb = nc.dram_tensor("tok_emb", [V, D], F32, kind="ExternalInput")
    pos_emb = nc.dram_tensor("pos_emb", [T, D], F32, kind="ExternalInput")
    Wq_f = nc.dram_tensor("Wq_f", [nlayers, D, HE], F32, kind="ExternalInput")
    Wk_f = nc.dram_tensor("Wk_f", [nlayers, D, HE], F32, kind="ExternalInput")
    Wv_f = nc.dram_tensor("Wv_f", [nlayers, D, HE], F32, kind="ExternalInput")
    Wproj = nc.dram_tensor("Wproj", [nlayers, HE, D], BF16, kind="ExternalInput")
    bproj = nc.dram_tensor("bproj", [nlayers, D], F32, kind="ExternalInput")
    ln1_g = nc.dram_tensor("ln1_g", [nlayers, D], F32, kind="ExternalInput")
    ln1_b = nc.dram_tensor("ln1_b", [nlayers, D], F32, kind="ExternalInput")
    ln2_g = nc.dram_tensor("ln2_g", [nlayers, D], F32, kind="ExternalInput")
    ln2_b = nc.dram_tensor("ln2_b", [nlayers, D], F32, kind="ExternalInput")
    W1 = nc.dram_tensor("W1", [nlayers, D, DFF], F32, kind="ExternalInput")
    b1 = nc.dram_tensor("b1", [nlayers, DFF], F32, kind="ExternalInput")
    W2 = nc.dram_tensor("W2", [nlayers, DFF, D], F32, kind="ExternalInput")
    b2 = nc.dram_tensor("b2", [nlayers, D], F32, kind="ExternalInput")
    lnf_g = nc.dram_tensor("lnf_g", [D], F32, kind="ExternalInput")
    lnf_b = nc.dram_tensor("lnf_b", [D], F32, kind="ExternalInput")
    Wout = nc.dram_tensor("Wout", [T * D, NC], F32, kind="ExternalInput")

    logits = nc.dram_tensor("logits", [BLOC, NC], F32, kind="ExternalOutput")
    tap = {}
    if taps:
        for nm, shp in (("h0T", [D, TLOC]), ("z1", [D, TLOC]), ("o", [HE, TLOC]),
                        ("h1", [D, TLOC]), ("hf", [TLOC, D])):
            tap[nm] = nc.dram_tensor("tap_" + nm, shp, F32, kind="ExternalOutput")

    with tile.TileContext(nc) as tc, nc.allow_low_precision(reason="float32r is full-width fp32 storage; PE rounds on read"):
        with (
            tc.tile_pool(name="persist", bufs=1) as persist,
            tc.tile_pool(name="qkv", bufs=1) as qkv,
            tc.tile_pool(name="vpool", bufs=1) as vpool,
            tc.tile_pool(name="expp", bufs=1) as expp,
            tc.tile_pool(name="wst", bufs=3) as wst,
            tc.tile_pool(name="chp", bufs=2) as chp,
            tc.tile_pool(name="single", bufs=1) as single,
            tc.tile_pool(name="misc", bufs=2) as misc,
            tc.tile_pool(name="pmm", bufs=3, space="PSUM") as pmm,
            tc.tile_pool(name="pacc", bufs=3, space="PSUM") as pacc,
            tc.tile_pool(name="pvec", bufs=2, space="PSUM") as pvec,
        ):
            hT = persist.tile([P, DO, TLOC], F32R, tag="hT")

            onesf = single.tile([P, P], F32, tag="onesf")
            nc.vector.memset(onesf[:], 1.0)
            ones_col = single.tile([P, 1], F32R, tag="ones_col")
            nc.vector.tensor_copy(ones_col[:], onesf[:, :1])
            ones128x = single.tile([P, P], F32R, tag="ones128x")
            nc.vector.tensor_copy(ones128x[:], onesf[:])
            rowbank = single.tile([P, TLOC], F32R, tag="rowbank")
            eps_col = single.tile([P, 1], F32, tag="eps_col")
            nc.vector.memset(eps_col[:], 1e-5)
            ident = single.tile([P, P], F32, tag="ident")
            make_identity(nc, ident[:])

            def dump_T(dst_dram, src, n_outer):
                """dump feature-major [P, n_outer, TLOC] tile to dram [n_outer*P, TLOC]"""
                dv = dst_dram.rearrange("(o p) t -> p o t", p=P)
                for dc in range(n_outer):
                    for th in range(2):
                        thc = slice(th * 512, (th + 1) * 512)
                        s = chp.tile([P, 512], F32, tag="ch", name="s")
                        nc.vector.tensor_copy(s[:], src[:, dc, thc])
                        nc.sync.dma_start(dv[:, dc, thc], s[:])

            # ================= Prologue: embed + transpose =================
            h_tok = persist.tile([P, DO, D], F32, tag="big")
            for o in range(DO):
                idx_t = misc.tile([P, 1], mybir.dt.int32, tag="idx")
                nc.sync.dma_start(idx_t[:], x_idx[o * P:(o + 1) * P, :])
                nc.gpsimd.indirect_dma_start(
                    out=h_tok[:, o, :], out_offset=None, in_=tok_emb[:],
                    in_offset=bass.IndirectOffsetOnAxis(ap=idx_t[:, :1], axis=0))
                to = o % (T // P)
                for dh in range(2):
                    dsl = slice(dh * 512, (dh + 1) * 512)
                    pos_t = chp.tile([P, 512], F32, tag="ch", name="pos_t")
                    nc.sync.dma_start(pos_t[:], pos_emb[to * P:(to + 1) * P, dsl])
                    nc.vector.tensor_add(h_tok[:, o, dsl], h_tok[:, o, dsl], pos_t[:])
            for o in range(DO):
                for dc in range(DO):
                    tp = pmm.tile([P, 512], F32, tag="mm")
                    nc.tensor.transpose(tp[:, :P], h_tok[:, o, dc * P:(dc + 1) * P],
                                        ident[:])
                    nc.vector.tensor_copy(hT[:, dc, o * P:(o + 1) * P], tp[:, :P])
            if taps:
                dump_T(tap["h0T"], hT, DO)

            # ---- transposed layernorm -------------------------------------
            def ln_transposed(src, g_dram, b_dram, dst):
                AB = persist.tile([P, 2, TLOC], F32R, tag="AB", name="AB")
                g_sb = misc.tile([P, DO], F32, tag="gb")
                nc.sync.dma_start(g_sb[:], g_dram.rearrange("(o p) -> p o", p=P))
                b_sb = misc.tile([P, DO], F32, tag="gb")
                nc.sync.dma_start(b_sb[:], b_dram.rearrange("(o p) -> p o", p=P))

                # rows on partitions: A@0, B@32 (rrow@64 used by attention)
                for th in range(2):
                    thc = slice(th * 512, (th + 1) * 512)
                    s1p = pvec.tile([1, 512], F32, tag="vec", name="s1p")
                    for do in range(DO):
                        nc.tensor.matmul(s1p[:], ones_col[:], src[:, do, thc],
                                         start=(do == 0), stop=(do == DO - 1))
                    s2p = pvec.tile([1, 512], F32, tag="vec", name="s2p")
                    for do in range(DO):
                        sq = chp.tile([P, 512], F32R, tag="ch", name="sq")
                        nc.vector.tensor_mul(sq[:], src[:, do, thc], src[:, do, thc])
                        nc.tensor.matmul(s2p[:], ones_col[:], sq[:],
                                         start=(do == 0), stop=(do == DO - 1))
                    a0 = rowbank[0:1, thc]    # mu, later B
                    b32 = rowbank[32:33, thc]  # var -> sd -> A
                    nc.vector.tensor_scalar_mul(a0, s1p[:], 1.0 / D)  # mu
                    nc.vector.scalar_tensor_tensor(
                        out=b32, in0=a0, scalar=-1.0, in1=a0,
                        op0=OP.mult, op1=OP.mult)                     # -mu^2
                    nc.vector.scalar_tensor_tensor(
                        out=b32, in0=s2p[:], scalar=1.0 / D, in1=b32,
                        op0=OP.mult, op1=OP.add)                      # var
                    nc.scalar.activation(b32, b32, AF.Sqrt, bias=eps_col[32:33, :])
                    nc.vector.reciprocal(b32, b32)                    # A = rstd
                    nc.vector.scalar_tensor_tensor(
                        out=a0, in0=s1p[:], scalar=-1.0 / D, in1=b32,
                        op0=OP.mult, op1=OP.mult)                     # B = -mu*A
                    for si, pb_ in ((0, 32), (1, 0)):
                        bp = pmm.tile([P, 512], F32, tag="mm")
                        nc.tensor.matmul(bp[:], ones128x[pb_:pb_ + 1, :],
                                         rowbank[pb_:pb_ + 1, thc],
                                         start=True, stop=True)
                        nc.vector.tensor_copy(AB[:, si, thc], bp[:])
                for do in range(DO):
                    for th in range(2):
                        thc = slice(th * 512, (th + 1) * 512)
                        t1 = chp.tile([P, 512], F32R, tag="ch", name="t1")
                        nc.vector.tensor_mul(t1[:], src[:, do, thc], AB[:, 0, thc])
                        nc.vector.tensor_add(t1[:], t1[:], AB[:, 1, thc])
                        nc.vector.tensor_scalar(
                            out=dst[:, do, thc], in0=t1[:],
                            scalar1=g_sb[:, do:do + 1], scalar2=b_sb[:, do:do + 1],
                            op0=OP.mult, op1=OP.add)

            # ========================= Layers ==============================
            for l in range(nlayers):
                Z = persist.tile([P, DO, TLOC], F32R, tag="Z")  # z1
                ln_transposed(hT, ln1_g[l], ln1_b[l], Z)
                if taps and l == 0:
                    dump_T(tap["z1"], Z, DO)

                oT = persist.tile([P, HO, TLOC], BF16, tag="AB")  # overlays AB (disjoint in time)
                for hg in range(H // 4):
                    hsl = slice(hg * 256, (hg + 1) * 256)
                    wq_st = wst.tile([P, DO, 256], F32R, tag="w")
                    nc.sync.dma_start(
                        wq_st[:], Wq_f[l].bitcast(F32R)
                        .rearrange("(o p) e -> p o e", p=P)[:, :, hsl])
                    wk_st = wst.tile([P, DO, 256], F32R, tag="w")
                    nc.sync.dma_start(
                        wk_st[:], Wk_f[l].bitcast(F32R)
                        .rearrange("(o p) e -> p o e", p=P)[:, :, hsl])
                    wv_st = wst.tile([P, DO, 256], F32R, tag="w")
                    nc.sync.dma_start(
                        wv_st[:], Wv_f[l].bitcast(F32R)
                        .rearrange("(o p) e -> p o e", p=P)[:, :, hsl])
                    for b in range(BLOC):
                        tsl = slice(b * T, (b + 1) * T)
                        q_t = qkv.tile([P, 2, T], F32R, tag="q")
                        k_t = qkv.tile([P, 2, T], F32R, tag="k")
                        for j in range(2):
                            jsl = slice(j * P, (j + 1) * P)
                            qp = pmm.tile([P, 512], F32, tag="mm")
                            for do in range(DO):
                                nc.tensor.matmul(qp[:], wq_st[:, do, jsl],
                                                 Z[:, do, tsl],
                                                 start=(do == 0), stop=(do == DO - 1))
                            nc.vector.tensor_copy(q_t[:, j, :], qp[:])
                            kp = pmm.tile([P, 512], F32, tag="mm")
                            for do in range(DO):
                                nc.tensor.matmul(kp[:], wk_st[:, do, jsl],
                                                 Z[:, do, tsl],
                                                 start=(do == 0), stop=(do == DO - 1))
                            nc.vector.tensor_copy(k_t[:, j, :], kp[:])
                        v_t = vpool.tile([P, 4, 256], F32R, tag="v")
                        for to in range(4):
                            vp = pmm.tile([P, 512], F32, tag="mm")
                            tc_sl = slice(b * T + to * P, b * T + (to + 1) * P)
                            for do in range(DO):
                                nc.tensor.matmul(vp[:, :256], Z[:, do, tc_sl],
                                                 wv_st[:, do, :],
                                                 start=(do == 0), stop=(do == DO - 1))
                            nc.vector.tensor_copy(v_t[:, to, :], vp[:, :256])
                        for hl in range(4):
                            h_glob = hg * 4 + hl
                            pb = (hl % 2) * 64
                            j = hl // 2
                            q_h = q_t[pb:pb + 64, j, :]
                            k_h = k_t[pb:pb + 64, j, :]
                            ex = expp.tile([P, 4, T], F32R, tag="exp")
                            for sc in range(4):
                                sp = pmm.tile([P, 512], F32, tag="mm")
                                nc.tensor.matmul(sp[:], k_h[:, sc * P:(sc + 1) * P],
                                                 q_h[:], start=True, stop=True)
                                nc.scalar.activation(ex[:, sc, :], sp[:], AF.Exp,
                                                     scale=float(HS ** -0.5))
                            ssum = pvec.tile([1, 512], F32, tag="vec")
                            for sc in range(4):
                                nc.tensor.matmul(ssum[:], ones_col[:], ex[:, sc, :],
                                                 start=(sc == 0), stop=(sc == 3))
                            hc = (h_glob % 2) * T
                            rrow = rowbank[64:65, hc:hc + T]
                            nc.vector.reciprocal(rrow, ssum[:])
                            rb = pacc.tile([64, 512], F32, tag="acc")
                            nc.tensor.matmul(rb[:], ones128x[64:65, :64], rrow,
                                             start=True, stop=True)
                            rb_sb = misc.tile([64, 512], F32R, tag="rbs")
                            nc.vector.tensor_copy(rb_sb[:], rb[:])
                            op_ = pacc.tile([64, 512], F32, tag="acc")
                            for sc in range(4):
                                nc.tensor.matmul(
                                    op_[:], v_t[:, sc, hl * 64:(hl + 1) * 64],
                                    ex[:, sc, :], start=(sc == 0), stop=(sc == 3))
                            heo = h_glob // 2
                            hep = (h_glob % 2) * 64
                            nc.vector.tensor_mul(
                                oT[hep:hep + 64, heo, b * T:(b + 1) * T],
                                op_[:], rb_sb[:])
                if taps and l == 0:
                    dump_T(tap["o"], oT, HO)

                # ---- proj + residual --------------------------------------
                bp_sb = misc.tile([P, DO], F32, tag="gb")
                nc.sync.dma_start(bp_sb[:], bproj[l].rearrange("(o p) -> p o", p=P))
                for dc in range(DO):
                    wp_st = wst.tile([P, HO, P], BF16, tag="w")
                    nc.sync.dma_start(
                        wp_st[:], Wproj[l]
                        .rearrange("(o p) d -> p o d", p=P)[:, :, dc * P:(dc + 1) * P])
                    for b in range(BLOC):
                        tsl = slice(b * T, (b + 1) * T)
                        pj = pmm.tile([P, 512], F32, tag="mm")
                        for ho in range(HO):
                            nc.tensor.matmul(pj[:], wp_st[:, ho, :], oT[:, ho, tsl],
                                             start=(ho == 0), stop=(ho == HO - 1))
                        nc.vector.scalar_tensor_tensor(
                            out=hT[:, dc, tsl], in0=pj[:],
                            scalar=bp_sb[:, dc:dc + 1], in1=hT[:, dc, tsl],
                            op0=OP.add, op1=OP.add)
                if taps and l == 0:
                    dump_T(tap["h1"], hT, DO)

                # ---- LN2 + FFN --------------------------------------------
                Z2 = persist.tile([P, DO, TLOC], F32R, tag="Z")
                ln_transposed(hT, ln2_g[l], ln2_b[l], Z2)
                b1_sb = misc.tile([P, FO], F32, tag="b1")
                nc.sync.dma_start(b1_sb[:], b1[l].rearrange("(o p) -> p o", p=P))
                b2_sb = misc.tile([P, DO], F32, tag="gb")
                nc.sync.dma_start(b2_sb[:], b2[l].rearrange("(o p) -> p o", p=P))
                w1v = W1[l].bitcast(F32R).rearrange("(o p) f -> p o f", p=P)
                w2v = W2[l].bitcast(F32R).rearrange("(o p) d -> p o d", p=P)
                for th in range(2):
                    tsl = slice(th * 512, (th + 1) * 512)
                    aT = persist.tile([P, FO, 512], F32R, tag="big")
                    for fc in range(FO):
                        w1_st = wst.tile([P, DO, P], F32R, tag="w")
                        nc.sync.dma_start(
                            w1_st[:], w1v[:, :, fc * P:(fc + 1) * P])
                        ap_ = pmm.tile([P, 512], F32, tag="mm")
                        for do in range(DO):
                            nc.tensor.matmul(ap_[:], w1_st[:, do, :], Z2[:, do, tsl],
                                             start=(do == 0), stop=(do == DO - 1))
                        nc.scalar.activation(aT[:, fc, :], ap_[:], AF.Gelu,
                                             bias=b1_sb[:, fc:fc + 1])
                    for dc in range(DO):
                        w2a = wst.tile([P, 16, P], F32R, tag="w")
                        nc.sync.dma_start(w2a[:], w2v[:, 0:16, dc * P:(dc + 1) * P])
                        w2b = wst.tile([P, 16, P], F32R, tag="w")
                        nc.sync.dma_start(w2b[:], w2v[:, 16:32, dc * P:(dc + 1) * P])
                        rp = pmm.tile([P, 512], F32, tag="mm")
                        for fc in range(FO):
                            wt = w2a if fc < 16 else w2b
                            nc.tensor.matmul(rp[:], wt[:, fc % 16, :], aT[:, fc, :],
                                             start=(fc == 0), stop=(fc == FO - 1))
                        nc.vector.scalar_tensor_tensor(
                            out=hT[:, dc, tsl], in0=rp[:],
                            scalar=b2_sb[:, dc:dc + 1], in1=hT[:, dc, tsl],
                            op0=OP.add, op1=OP.add)

            # ================= Epilogue ====================================
            h_tok2 = persist.tile([P, DO, D], F32, tag="big")
            for o in range(DO):
                for dc in range(DO):
                    tp = pmm.tile([P, 512], F32, tag="mm")
                    nc.tensor.transpose(tp[:, :P],
                                        hT[:, dc, o * P:(o + 1) * P].bitcast(F32),
                                        ident[:])
                    nc.vector.tensor_copy(h_tok2[:, o, dc * P:(dc + 1) * P], tp[:, :P])
            gB = wst.tile([P, D], F32, tag="w", name="gB")
            nc.sync.dma_start(gB[:], bass.AP(tensor=lnf_g, offset=0,
                                             ap=[[0, P], [1, D]]))
            bB = wst.tile([P, D], F32, tag="w", name="bB")
            nc.sync.dma_start(bB[:], bass.AP(tensor=lnf_b, offset=0,
                                             ap=[[0, P], [1, D]]))
            for o in range(DO):
                st1 = misc.tile([P, 2, nc.vector.BN_STATS_DIM], F32, tag="bst")
                for sg in range(2):
                    nc.vector.bn_stats(st1[:, sg, :],
                                       h_tok2[:, o, sg * 512:(sg + 1) * 512])
                mv = misc.tile([P, nc.vector.BN_AGGR_DIM], F32, tag="bag")
                nc.vector.bn_aggr(mv[:], st1[:])
                sd = misc.tile([P, 1], F32, tag="sd")
                nc.scalar.activation(sd[:], mv[:, 1:2], AF.Sqrt, bias=eps_col[:])
                nc.vector.reciprocal(sd[:], sd[:])
                nc.vector.tensor_scalar(
                    out=h_tok2[:, o, :], in0=h_tok2[:, o, :],
                    scalar1=mv[:, 0:1], scalar2=sd[:],
                    op0=OP.subtract, op1=OP.mult)
                nc.vector.tensor_mul(h_tok2[:, o, :], h_tok2[:, o, :], gB[:])
                nc.vector.tensor_add(h_tok2[:, o, :], h_tok2[:, o, :], bB[:])
            if taps:
                dv = tap["hf"].rearrange("(o p) d -> p o d", p=P)
                for o in range(DO):
                    for th in range(2):
                        thc = slice(th * 512, (th + 1) * 512)
                        s = chp.tile([P, 512], F32, tag="ch", name="s")
                        nc.vector.tensor_copy(s[:], h_tok2[:, o, thc])
                        nc.sync.dma_start(dv[:, o, thc], s[:])

            # unembed: logits[b, c] = sum_{t,d} h[b,t,d] * Wout[t*D+d, c]
            parts = misc.tile([P, 8 * BLOC * NC], F32, tag="parts")
            w3v = Wout.rearrange("(t d) c -> t d c", d=D) \
                      .rearrange("(tp p) d c -> p tp (d c)", p=P)  # [P, 4, D*NC]
            for tp_ in range(2):
                wz = persist.tile([P, 2, D * NC], F32, tag="Z")
                nc.sync.dma_start(wz[:], w3v[:, tp_ * 2:tp_ * 2 + 2, :])
                wzv = wz.rearrange("p a (d c) -> p a d c", c=NC)
                for i in range(2):
                    for b in range(BLOC):
                        for c in range(NC):
                            for dh in range(2):
                                dsl = slice(dh * 512, (dh + 1) * 512)
                                junk = chp.tile([P, 512], F32, tag="ch", name="junk")
                                col = (((tp_ * 2 + i) * 2 + dh) * (BLOC * NC)
                                       + b * NC + c)
                                nc.vector.scalar_tensor_tensor(
                                    out=junk[:], in0=wzv[:, i, dsl, c], scalar=1.0,
                                    in1=h_tok2[:, b * 4 + tp_ * 2 + i, dsl],
                                    op0=OP.mult, op1=OP.mult,
                                    accum_out=parts[:, col:col + 1])
            p2 = misc.tile([P, BLOC * NC], F32, tag="p2")
            nc.vector.tensor_add(p2[:], parts[:, 0:BLOC * NC],
                                 parts[:, BLOC * NC:2 * BLOC * NC])
            for g_ in range(2, 8):
                nc.vector.tensor_add(
                    p2[:], p2[:],
                    parts[:, g_ * BLOC * NC:(g_ + 1) * BLOC * NC])
            outp = pvec.tile([BLOC * NC, 1], F32, tag="vec")
            nc.tensor.matmul(outp[:], p2[:], onesf[:, :1], start=True, stop=True)
            outs = misc.tile([BLOC * NC, 1], F32, tag="outs")
            nc.vector.tensor_copy(outs[:], outp[:])
            nc.sync.dma_start(logits.rearrange("b c -> (b c)")[:, None], outs[:])

    nc.compile()
    return nc


_NC_CACHE = {}


def _get_nc():
    key = (NLAYERS, TAPS)
    if key not in _NC_CACHE:
        _NC_CACHE[key] = _build()
    return _NC_CACHE[key]


def kernel(**inputs):
    global LAST_RESULTS
    inp = {k: np.asarray(v) for k, v in inputs.items()}
    L_ = NLAYERS
    x = inp["x"].astype(np.int32)
    wq = np.ascontiguousarray(inp["Wq"][:L_].transpose(0, 2, 1, 3).reshape(L_, D, HE))
    wk = np.ascontiguousarray(inp["Wk"][:L_].transpose(0, 2, 1, 3).reshape(L_, D, HE))
    wv = np.ascontiguousarray(inp["Wv"][:L_].transpose(0, 2, 1, 3).reshape(L_, D, HE))
    f32c = lambda a: np.ascontiguousarray(a, dtype=np.float32)
    common = dict(
        tok_emb=f32c(inp["tok_emb"]), pos_emb=f32c(inp["pos_emb"][:T]),
        Wq_f=f32c(wq), Wk_f=f32c(wk), Wv_f=f32c(wv),
        Wproj=np.ascontiguousarray(inp["Wproj"][:L_]).astype(ml_dtypes.bfloat16), bproj=f32c(inp["bproj"][:L_]),
        ln1_g=f32c(inp["ln1_g"][:L_]), ln1_b=f32c(inp["ln1_b"][:L_]),
        ln2_g=f32c(inp["ln2_g"][:L_]), ln2_b=f32c(inp["ln2_b"][:L_]),
        W1=f32c(inp["W1"][:L_]), b1=f32c(inp["b1"][:L_]),
        W2=f32c(inp["W2"][:L_]), b2=f32c(inp["b2"][:L_]),
        lnf_g=f32c(inp["lnf_g"]), lnf_b=f32c(inp["lnf_b"]),
        Wout=f32c(inp["Wout"]),
    )
    in_maps = []
    for c in range(NCORES):
        m = dict(common)
        m["x_idx"] = np.ascontiguousarray(
            x[c * BLOC:(c + 1) * BLOC].reshape(TLOC, 1))
        in_maps.append(m)

    nc = _get_nc()
    res = run_bass_kernel_spmd(nc, in_maps, core_ids=list(range(NCORES)))
    LAST_RESULTS = res
    out = np.concatenate([res.results[c]["logits"] for c in range(NCORES)], axis=0)
    # bout is a zero-init bias in this model; adding it on host is part of the
    # gather/unshard glue (it is a [3]-vector add on the [16,3] output).
    out = out + np.asarray(inp["bout"], np.float32)[None, :]
    return out.astype(np.float32)

